# revision 12
# baseline (speedup 1.0000x reference)
"""PCEN (per-channel energy normalization) Trainium2 Bass kernel, v2.

Problem: data [1024, 50000] f32, EMA along time (s=0.5) then
    out = (x / (EPS + M)**alpha + delta)**r - delta**r

Sharding: freq axis (dim 0) split across 8 NeuronCores, 128 rows/core.

v2 design (vs the v1 recip+table-switch pipeline):
  - With v2 = 2M from the native scan, q = x/v2 is computed by an exact
    DVE tensor_tensor divide (q <= ~1 always since v2 >= x), and
    u = x/(eps+M)^alpha = q * w(v2) where w(v2) = v2*(eps+v2/2)^-alpha
    spans only [1.66, 2.0]: affine-in-int16-bits fit, 0.3% max rel err
    over the steady-state v2 range [2e-3, 2.2].
  - No ACT Reciprocal at all -> zero activation-table switches; ACT runs
    Sqrt (+ Copy for part of the w fits) from one table set.
  - Output is written as uint8: o8 = trunc(K*sqrt(u+delta)) via a single
    pre-scaled ACT Sqrt (scale=K^2, bias=delta*K^2), K=127. The host
    decodes out = (o8+0.5)/K - delta^r. Quant err 0.5/127 = 3.9e-3 abs,
    well inside the 2e-2 budget, and it halves output DMA traffic.
  - Scans are made independent per tile with a 16-col zero-seeded halo
    (EMA forgets at 2^-t; the halo error is ~v2*2^-17), so the scan can
    be split across Pool (18 tiles) and DVE (3 tiles) with no serial
    carry chain.
  - Tile 0 (512 cols, where v2 can legitimately be tiny) runs an exact
    eps path: v = 0.5*v2 + eps in fp32, q0 = x/v (DVE divide), and
    g0 = v^(1-alpha) fitted affinely in the int32 bits of fp32 v over
    [5e-7, 1.2] (the 0.02 exponent makes this ~1% accurate, and u<=2
    there, so the output error stays ~2e-3).

Engine balance (per-core, TimelineSim rates): Pool 66us scan; DVE 67us
(scan 4 + div 27 + mult 27 + ~26k cols of w-fit at 4x); ACT 67us (sqrt
45 + ~23k cols of w-fit via Copy); DMA 58us (fp16 in + uint8 out).
"""

import numpy as np

import concourse.bass as bass
import concourse.bacc as bacc
import concourse.mybir as mybir
from concourse import tile
from concourse.bass_utils import run_bass_kernel_spmd

F, T = 1024, 50000
NCORES = 8
FP = F // NCORES  # 128 partitions per core
EPS = 1e-6
K = 127.0  # uint8 output scale

T0 = 512           # tile 0: exact-eps path
TC = 2560          # max steady tile width (buffer size)
HALO = 16          # zero-seeded scan warm-up cols per steady tile

# Schedule knobs (tuned against TimelineSim):
HEAD = (512, 1024)        # tile widths at the start (incl tile 0)
TAIL = (1024, 768, 512, 256)  # tile widths at the end
DVE_SCAN = (2, 3)         # steady tiles scanned on DVE (rest Pool)
ACT_G_STEP = 2            # every ACT_G_STEP-th steady tile's w-fit on ACT
ACT_G_MAX = 10            # number of w-fits on ACT
DMA_ORDER = (1, 0, 4, 2, 3)   # upfront in-DMA issue order

_CACHE: dict = {}


def _tiles():
    mid = T - sum(HEAD) - sum(TAIL)
    n_mid = max(1, round(mid / TC + 0.499))
    base = mid // n_mid
    rem = mid - base * n_mid
    mids = [base + (1 if i < rem else 0) for i in range(n_mid)]
    assert all(m + HALO <= TC + HALO for m in mids)
    tiles = list(HEAD) + mids + list(TAIL)
    assert sum(tiles) == T
    return tiles


def _irls_fit(codes, target):
    """Minimax-relative affine fit target ~ c1*codes + c0 via IRLS."""
    w = np.ones_like(target)
    co = np.polyfit(codes, target, 1, w=w / target)
    for _ in range(80):
        co = np.polyfit(codes, target, 1, w=w / target)
        rel = (np.polyval(co, codes) - target) / target
        w = (np.abs(rel) + 1e-7) * w
        w /= w.max()
    return float(co[0]), float(co[1])


def _fit_w_steady(alpha: float):
    """w(v2) = v2*(eps+v2/2)^-alpha, affine in int16 bits of fp16 v2,
    over the steady-state range [2e-3, 2.2]."""
    lo = np.float16(2e-3).view(np.int16)
    hi = np.float16(2.2).view(np.int16)
    codes = np.arange(int(lo), int(hi) + 1, dtype=np.int16)
    vals = codes.view(np.float16).astype(np.float64)
    keep = (vals > 0) & np.isfinite(vals)
    bc = codes[keep].astype(np.float64)
    vals = vals[keep]
    wi = vals * (EPS + 0.5 * vals) ** (-alpha)
    return _irls_fit(bc, wi)


def _fit_g0_tile0(alpha: float):
    """g0(v) = v^(1-alpha), affine in int32 bits of fp32 v over
    [5e-7, 1.2] (tile-0 exact-eps path; v = 0.5*v2 + eps)."""
    v = np.geomspace(5e-7, 1.2, 20000).astype(np.float32)
    bc = v.view(np.int32).astype(np.float64)
    gi = v.astype(np.float64) ** (1.0 - alpha)
    return _irls_fit(bc, gi)


def _build(alpha: float, r: float, delta: float):
    dt = mybir.dt
    Act = mybir.ActivationFunctionType
    Alu = mybir.AluOpType
    use_sqrt = abs(r - 0.5) < 1e-12
    c1, c0 = _fit_w_steady(alpha)
    d1, d0 = _fit_g0_tile0(alpha)
    k2 = float(K * K)

    nc = bacc.Bacc("TRN2", debug=False, enable_asserts=False,
                   target_bir_lowering=False)
    x = nc.dram_tensor("x", [FP, T], dt.float16, kind="ExternalInput").ap()
    y = nc.dram_tensor("y", [FP, T], dt.uint8, kind="ExternalOutput").ap()

    tiles = _tiles()
    N = len(tiles)
    offs = [0]
    for w in tiles:
        offs.append(offs[-1] + w)

    def scan_on_pool(k):
        return k >= 1 and k not in DVE_SCAN

    def g_on_act(k):
        # spread ACT-g tiles across the run (keep the tail on DVE)
        if k == 0:
            return False
        return ((k - 1) % ACT_G_STEP == 0) and \
            ((k - 1) // ACT_G_STEP < ACT_G_MAX)

    with tile.TileContext(nc) as tc:
        with (
            tc.tile_pool(name="const", bufs=1) as cpool,
            tc.tile_pool(name="x", bufs=8) as xpool,
            tc.tile_pool(name="m", bufs=6) as mpool,
            tc.tile_pool(name="g", bufs=5) as gpool,
            tc.tile_pool(name="o", bufs=5) as opool,
            tc.tile_pool(name="t0", bufs=1) as t0pool,
        ):
            halfful = cpool.tile([FP, TC + HALO], dt.float16, tag="half")
            nc.gpsimd.memset(halfful[:], 0.5)
            bias_t = cpool.tile([FP, 1], dt.float32, tag="bias")
            nc.gpsimd.memset(bias_t[:], float(delta) * k2)
            # Warm-up Sqrt with no data deps: pulls the one ACT table load
            # into the ramp.
            warm = cpool.tile([FP, 1], dt.float32, tag="warm")
            nc.scalar.activation(warm[:], bias_t[:],
                                 Act.Sqrt if use_sqrt else Act.Ln,
                                 bias=bias_t[:], scale=1.0)

            st: list[dict] = [dict() for _ in range(N)]

            def dma_in(k):
                w = tiles[k]
                if k == 0:
                    xt = xpool.tile([FP, TC + HALO], dt.float16, tag="x")
                    nc.sync.dma_start(xt[:, :w], x[:, :w])
                else:
                    xt = xpool.tile([FP, TC + HALO], dt.float16, tag="x")
                    nc.sync.dma_start(xt[:, :w + HALO],
                                      x[:, offs[k] - HALO:offs[k] + w])
                st[k]["x"] = xt

            def scan(k):
                w = tiles[k]
                wh = w if k == 0 else w + HALO
                m2 = mpool.tile([FP, TC + HALO], dt.float16, tag="m")
                eng = nc.gpsimd if scan_on_pool(k) else nc.vector
                eng.tensor_tensor_scan(
                    m2[:, :wh], halfful[:, :wh], st[k]["x"][:, :wh],
                    0.0, Alu.mult, Alu.add)
                st[k]["m"] = m2

            def tile0_mid():
                w = tiles[0]
                xt, m2 = st[0]["x"], st[0]["m"]
                vf = t0pool.tile([FP, T0], dt.float32, tag="v0")
                nc.vector.tensor_scalar(vf[:, :w], m2[:, :w], 0.5, EPS,
                                        op0=Alu.mult, op1=Alu.add)
                q = xt  # reuse x tile for q
                nc.vector.tensor_tensor(q[:, :w], xt[:, :w], vf[:, :w],
                                        Alu.divide)
                g = gpool.tile([FP, TC], dt.float16, tag="g")
                b32 = vf[:, :w].bitcast(dt.int32)
                nc.vector.tensor_scalar(g[:, :w], b32, d1, d0,
                                        op0=Alu.mult, op1=Alu.add)
                u = m2  # reuse m2 tile for u
                nc.vector.tensor_tensor(u[:, :w], q[:, :w], g[:, :w],
                                        Alu.mult)
                st[0]["u"] = u

            def mid_qg(k):
                w = tiles[k]
                xt, m2 = st[k]["x"], st[k]["m"]
                xs = xt[:, HALO:HALO + w]
                v2 = m2[:, HALO:HALO + w]
                q = xs  # reuse x tile for q (in place)
                nc.vector.tensor_tensor(q, xs, v2, Alu.divide)
                g = gpool.tile([FP, TC], dt.float16, tag="g")
                b16 = v2.bitcast(dt.int16)
                if g_on_act(k):
                    nc.scalar.activation(g[:, :w], b16, Act.Copy,
                                         bias=c0, scale=c1)
                else:
                    nc.vector.tensor_scalar(g[:, :w], b16, c1, c0,
                                            op0=Alu.mult, op1=Alu.add)
                st[k]["q"] = q
                st[k]["g"] = g

            def mid_u(k):
                w = tiles[k]
                g = st[k]["g"]
                u = g  # reuse g tile for u
                nc.vector.tensor_tensor(u[:, :w], st[k]["q"], g[:, :w],
                                        Alu.mult)
                st[k]["u"] = u

            def tail(k):
                w = tiles[k]
                u = st[k]["u"]
                o8 = opool.tile([FP, TC], dt.uint8, tag="o")
                if use_sqrt:
                    # o8 = trunc(K*sqrt(u+delta)) = trunc(sqrt(K^2*u+K^2*d))
                    nc.scalar.activation(o8[:, :w], u[:, :w], Act.Sqrt,
                                         bias=bias_t[:], scale=k2)
                else:
                    sf = gpool.tile([FP, TC], dt.float16, tag="g")
                    nc.scalar.activation(sf[:, :w], u[:, :w], Act.Ln,
                                         bias=bias_t[:], scale=k2)
                    nc.scalar.activation(o8[:, :w], sf[:, :w], Act.Exp,
                                         scale=float(r))
                nc.sync.dma_start(y[:, offs[k]:offs[k] + w], o8[:, :w])
                st[k].clear()

            # Software-pipelined emission: u runs one tile behind q/g and
            # sqrt+dma_out two tiles behind, so every op's inputs are ready
            # well before its engine dispatches it (no cross-engine
            # round-trip stalls in the in-order queues). DVE's share of the
            # steady scans is front-loaded into the ramp, where DVE would
            # otherwise idle; the tail then ends on Pool's last (small)
            # scan with a short drain chain.
            for j in DMA_ORDER:
                if j < N:
                    dma_in(j)
            scan(0)
            tile0_mid()  # q0, g0, u0 in one go
            for k in DVE_SCAN:
                if k < N:
                    scan(k)
            for k in range(1, N):
                if scan_on_pool(k):
                    scan(k)
                if k + 4 < N:
                    dma_in(k + 4)
                mid_qg(k)
                if k >= 2:
                    mid_u(k - 1)
                if k >= 2:
                    tail(k - 2)
            mid_u(N - 1)
            tail(N - 2)
            tail(N - 1)

    nc.compile()
    return nc


def _get_nc(alpha: float, r: float, delta: float):
    key = (round(alpha, 9), round(r, 9), round(delta, 9))
    if key not in _CACHE:
        _CACHE[key] = _build(alpha, r, delta)
    return _CACHE[key]


def _decode(o8: np.ndarray, r: float, delta: float) -> np.ndarray:
    return (o8.astype(np.float32) + np.float32(0.5)) / np.float32(K) \
        - np.float32(float(delta) ** float(r))


def _make_runner(nc):
    """Cached variant of bass2jax.run_bass_via_pjrt's multi-core branch.

    run_bass_kernel_spmd builds a fresh jax.jit closure per call (full
    retrace) and round-trips the full array through per-core split +
    concat. Since the 8 shards concatenated on axis 0 ARE the full
    [1024, 50000] array, we jit once and feed/return the full array
    directly.
    """
    import jax
    from jax.experimental.shard_map import shard_map
    from jax.sharding import Mesh, PartitionSpec
    from concourse import bass2jax

    bass2jax.install_neuronx_cc_hook()
    if nc.dbg_callbacks:
        raise RuntimeError("dbg callbacks unsupported in cached runner")
    partition_name = (nc.partition_id_tensor.name
                      if nc.partition_id_tensor else None)
    in_names, out_names, out_avals = [], [], []
    for alloc in nc.m.functions[0].allocations:
        if not isinstance(alloc, mybir.MemoryLocationSet):
            continue
        name = alloc.memorylocations[0].name
        if alloc.kind == "ExternalInput":
            if name != partition_name:
                in_names.append(name)
        elif alloc.kind == "ExternalOutput":
            out_names.append(name)
            out_avals.append(jax.core.ShapedArray(
                tuple(alloc.tensor_shape), mybir.dt.np(alloc.dtype)))
    extra_ins = {}
    if nc.dbg_addr is not None:
        extra_ins[nc.dbg_addr.name] = np.zeros((1, 2), np.uint32)
        if nc.dbg_addr.name not in in_names:
            in_names.append(nc.dbg_addr.name)
    assert in_names[0] == "x" and out_names == ["y"], (in_names, out_names)
    n_params = len(in_names)
    all_names = list(in_names) + list(out_names)
    if partition_name is not None:
        all_names.append(partition_name)
    donate = tuple(range(n_params, n_params + len(out_names)))

    def _body(*args):
        operands = list(args)
        if partition_name is not None:
            operands.append(bass2jax.partition_id_tensor())
        outs = bass2jax._bass_exec_p.bind(
            *operands,
            out_avals=tuple(out_avals),
            in_names=tuple(all_names),
            out_names=tuple(out_names),
            lowering_input_output_aliases=(),
            sim_require_finite=True,
            sim_require_nnan=True,
            nc=nc,
        )
        return tuple(outs)

    devices = jax.devices()[:NCORES]
    assert len(devices) == NCORES, devices
    mesh = Mesh(np.asarray(devices), ("core",))
    nio = n_params + len(out_names)
    sharded = jax.jit(
        shard_map(_body, mesh=mesh,
                  in_specs=(PartitionSpec("core"),) * nio,
                  out_specs=(PartitionSpec("core"),) * len(out_names),
                  check_rep=False),
        donate_argnums=donate, keep_unused=True)

    def run(data: np.ndarray) -> np.ndarray:
        extras = [np.concatenate([v] * NCORES, axis=0)
                  for v in extra_ins.values()]
        zeros = [np.zeros((NCORES * a.shape[0], *a.shape[1:]), a.dtype)
                 for a in out_avals]
        outs = sharded(data, *extras, *zeros)
        return np.asarray(outs[0])

    return run


def kernel(data, alpha=None, r=None, delta=None) -> np.ndarray:
    data = np.asarray(data)
    assert data.shape == (F, T), data.shape
    dh = np.ascontiguousarray(data.astype(np.float16))
    a = float(np.asarray(alpha).reshape(-1)[0]) if alpha is not None else 0.98
    rr = float(np.asarray(r).reshape(-1)[0]) if r is not None else 0.5
    d = float(np.asarray(delta).reshape(-1)[0]) if delta is not None else 2.0

    nc = _get_nc(a, rr, d)
    rkey = ("runner", round(a, 9), round(rr, 9), round(d, 9))
    try:
        if rkey not in _CACHE:
            _CACHE[rkey] = _make_runner(nc)
        o8 = _CACHE[rkey](dh)
    except Exception:  # fall back to the stock SPMD path
        _CACHE[rkey] = None
        in_maps = [{"x": dh[i * FP:(i + 1) * FP]} for i in range(NCORES)]
        res = run_bass_kernel_spmd(nc, in_maps, core_ids=list(range(NCORES)))
        o8 = np.concatenate([res.results[i]["y"] for i in range(NCORES)],
                            axis=0)
    return _decode(o8, rr, d)


# revision 13
# speedup vs baseline: 1.0487x; 1.0487x over previous
"""PCEN (per-channel energy normalization) Trainium2 Bass kernel, v2.

Problem: data [1024, 50000] f32, EMA along time (s=0.5) then
    out = (x / (EPS + M)**alpha + delta)**r - delta**r

Sharding: freq axis (dim 0) split across 8 NeuronCores, 128 rows/core.

v2 design (vs the v1 recip+table-switch pipeline):
  - With v2 = 2M from the native scan, q = x/v2 is computed by an exact
    DVE tensor_tensor divide (q <= ~1 always since v2 >= x), and
    u = x/(eps+M)^alpha = q * w(v2) where w(v2) = v2*(eps+v2/2)^-alpha
    spans only [1.66, 2.0]: affine-in-int16-bits fit, 0.3% max rel err
    over the steady-state v2 range [2e-3, 2.2].
  - No ACT Reciprocal at all -> zero activation-table switches; ACT runs
    Sqrt (+ Copy for part of the w fits) from one table set.
  - Output is written as uint8: o8 = trunc(K*sqrt(u+delta)) via a single
    pre-scaled ACT Sqrt (scale=K^2, bias=delta*K^2), K=127. The host
    decodes out = (o8+0.5)/K - delta^r. Quant err 0.5/127 = 3.9e-3 abs,
    well inside the 2e-2 budget, and it halves output DMA traffic.
  - Scans are made independent per tile with a 16-col zero-seeded halo
    (EMA forgets at 2^-t; the halo error is ~v2*2^-17), so the scan can
    be split across Pool (18 tiles) and DVE (3 tiles) with no serial
    carry chain.
  - Tile 0 (512 cols, where v2 can legitimately be tiny) runs an exact
    eps path: v = 0.5*v2 + eps in fp32, q0 = x/v (DVE divide), and
    g0 = v^(1-alpha) fitted affinely in the int32 bits of fp32 v over
    [5e-7, 1.2] (the 0.02 exponent makes this ~1% accurate, and u<=2
    there, so the output error stays ~2e-3).

Engine balance (per-core, TimelineSim rates): Pool 66us scan; DVE 67us
(scan 4 + div 27 + mult 27 + ~26k cols of w-fit at 4x); ACT 67us (sqrt
45 + ~23k cols of w-fit via Copy); DMA 58us (fp16 in + uint8 out).
"""

import numpy as np

import concourse.bass as bass
import concourse.bacc as bacc
import concourse.mybir as mybir
from concourse import tile
from concourse.bass_utils import run_bass_kernel_spmd

F, T = 1024, 50000
NCORES = 8
FP = F // NCORES  # 128 partitions per core
EPS = 1e-6
K = 127.0  # uint8 output scale

T0 = 512           # tile 0: exact-eps path
TC = 2560          # max steady tile width (buffer size)
HALO = 16          # zero-seeded scan warm-up cols per steady tile

# Schedule knobs (tuned against TimelineSim):
HEAD = (512, 1024)        # tile widths at the start (incl tile 0)
TAIL = (1536, 1024, 512)  # tile widths at the end
DVE_SCAN = (2, 3)         # steady tiles scanned on DVE (rest Pool)
ACT_G_STEP = 2            # every ACT_G_STEP-th steady tile's w-fit on ACT
ACT_G_MAX = 8             # number of w-fits on ACT
DMA_ORDER = (1, 0, 4, 2, 3)   # upfront in-DMA issue order

_CACHE: dict = {}


def _tiles():
    mid = T - sum(HEAD) - sum(TAIL)
    n_mid = max(1, round(mid / TC + 0.499))
    base = mid // n_mid
    rem = mid - base * n_mid
    mids = [base + (1 if i < rem else 0) for i in range(n_mid)]
    assert all(m + HALO <= TC + HALO for m in mids)
    tiles = list(HEAD) + mids + list(TAIL)
    assert sum(tiles) == T
    return tiles


def _irls_fit(codes, target):
    """Minimax-relative affine fit target ~ c1*codes + c0 via IRLS."""
    w = np.ones_like(target)
    co = np.polyfit(codes, target, 1, w=w / target)
    for _ in range(80):
        co = np.polyfit(codes, target, 1, w=w / target)
        rel = (np.polyval(co, codes) - target) / target
        w = (np.abs(rel) + 1e-7) * w
        w /= w.max()
    return float(co[0]), float(co[1])


def _fit_w_steady(alpha: float):
    """w(v2) = v2*(eps+v2/2)^-alpha, affine in int16 bits of fp16 v2,
    over the steady-state range [2e-3, 2.2]."""
    lo = np.float16(2e-3).view(np.int16)
    hi = np.float16(2.2).view(np.int16)
    codes = np.arange(int(lo), int(hi) + 1, dtype=np.int16)
    vals = codes.view(np.float16).astype(np.float64)
    keep = (vals > 0) & np.isfinite(vals)
    bc = codes[keep].astype(np.float64)
    vals = vals[keep]
    wi = vals * (EPS + 0.5 * vals) ** (-alpha)
    return _irls_fit(bc, wi)


def _fit_g0_tile0(alpha: float):
    """g0(v) = v^(1-alpha), affine in int32 bits of fp32 v over
    [5e-7, 1.2] (tile-0 exact-eps path; v = 0.5*v2 + eps)."""
    v = np.geomspace(5e-7, 1.2, 20000).astype(np.float32)
    bc = v.view(np.int32).astype(np.float64)
    gi = v.astype(np.float64) ** (1.0 - alpha)
    return _irls_fit(bc, gi)


def _build(alpha: float, r: float, delta: float):
    dt = mybir.dt
    Act = mybir.ActivationFunctionType
    Alu = mybir.AluOpType
    use_sqrt = abs(r - 0.5) < 1e-12
    c1, c0 = _fit_w_steady(alpha)
    d1, d0 = _fit_g0_tile0(alpha)
    k2 = float(K * K)

    nc = bacc.Bacc("TRN2", debug=False, enable_asserts=False,
                   target_bir_lowering=False)
    x = nc.dram_tensor("x", [FP, T], dt.float16, kind="ExternalInput").ap()
    y = nc.dram_tensor("y", [FP, T], dt.uint8, kind="ExternalOutput").ap()

    tiles = _tiles()
    N = len(tiles)
    offs = [0]
    for w in tiles:
        offs.append(offs[-1] + w)

    def scan_on_pool(k):
        return k >= 1 and k not in DVE_SCAN

    def g_on_act(k):
        # spread ACT-g tiles across the run (keep the tail on DVE)
        if k == 0:
            return False
        return ((k - 1) % ACT_G_STEP == 0) and \
            ((k - 1) // ACT_G_STEP < ACT_G_MAX)

    with tile.TileContext(nc) as tc:
        with (
            tc.tile_pool(name="const", bufs=1) as cpool,
            tc.tile_pool(name="x", bufs=8) as xpool,
            tc.tile_pool(name="m", bufs=6) as mpool,
            tc.tile_pool(name="g", bufs=5) as gpool,
            tc.tile_pool(name="o", bufs=5) as opool,
            tc.tile_pool(name="t0", bufs=1) as t0pool,
        ):
            halfful = cpool.tile([FP, TC + HALO], dt.float16, tag="half")
            nc.gpsimd.memset(halfful[:], 0.5)
            bias_t = cpool.tile([FP, 1], dt.float32, tag="bias")
            nc.gpsimd.memset(bias_t[:], float(delta) * k2)
            # Warm-up Sqrt with no data deps: pulls the one ACT table load
            # into the ramp.
            warm = cpool.tile([FP, 1], dt.float32, tag="warm")
            nc.scalar.activation(warm[:], bias_t[:],
                                 Act.Sqrt if use_sqrt else Act.Ln,
                                 bias=bias_t[:], scale=1.0)

            st: list[dict] = [dict() for _ in range(N)]

            def dma_in(k):
                w = tiles[k]
                if k == 0:
                    xt = xpool.tile([FP, TC + HALO], dt.float16, tag="x")
                    nc.sync.dma_start(xt[:, :w], x[:, :w])
                else:
                    xt = xpool.tile([FP, TC + HALO], dt.float16, tag="x")
                    nc.sync.dma_start(xt[:, :w + HALO],
                                      x[:, offs[k] - HALO:offs[k] + w])
                st[k]["x"] = xt

            def scan(k):
                w = tiles[k]
                wh = w if k == 0 else w + HALO
                m2 = mpool.tile([FP, TC + HALO], dt.float16, tag="m")
                eng = nc.gpsimd if scan_on_pool(k) else nc.vector
                eng.tensor_tensor_scan(
                    m2[:, :wh], halfful[:, :wh], st[k]["x"][:, :wh],
                    0.0, Alu.mult, Alu.add)
                st[k]["m"] = m2

            def tile0_mid():
                w = tiles[0]
                xt, m2 = st[0]["x"], st[0]["m"]
                vf = t0pool.tile([FP, T0], dt.float32, tag="v0")
                nc.vector.tensor_scalar(vf[:, :w], m2[:, :w], 0.5, EPS,
                                        op0=Alu.mult, op1=Alu.add)
                q = xt  # reuse x tile for q
                nc.vector.tensor_tensor(q[:, :w], xt[:, :w], vf[:, :w],
                                        Alu.divide)
                g = gpool.tile([FP, TC], dt.float16, tag="g")
                b32 = vf[:, :w].bitcast(dt.int32)
                nc.vector.tensor_scalar(g[:, :w], b32, d1, d0,
                                        op0=Alu.mult, op1=Alu.add)
                u = m2  # reuse m2 tile for u
                nc.vector.tensor_tensor(u[:, :w], q[:, :w], g[:, :w],
                                        Alu.mult)
                st[0]["u"] = u

            def mid_qg(k):
                w = tiles[k]
                xt, m2 = st[k]["x"], st[k]["m"]
                xs = xt[:, HALO:HALO + w]
                v2 = m2[:, HALO:HALO + w]
                q = xs  # reuse x tile for q (in place)
                nc.vector.tensor_tensor(q, xs, v2, Alu.divide)
                g = gpool.tile([FP, TC], dt.float16, tag="g")
                b16 = v2.bitcast(dt.int16)
                if g_on_act(k):
                    nc.scalar.activation(g[:, :w], b16, Act.Copy,
                                         bias=c0, scale=c1)
                else:
                    nc.vector.tensor_scalar(g[:, :w], b16, c1, c0,
                                            op0=Alu.mult, op1=Alu.add)
                st[k]["q"] = q
                st[k]["g"] = g

            def mid_u(k):
                w = tiles[k]
                g = st[k]["g"]
                u = g  # reuse g tile for u
                nc.vector.tensor_tensor(u[:, :w], st[k]["q"], g[:, :w],
                                        Alu.mult)
                st[k]["u"] = u

            def tail(k):
                w = tiles[k]
                u = st[k]["u"]
                o8 = opool.tile([FP, TC], dt.uint8, tag="o")
                if use_sqrt:
                    # o8 = trunc(K*sqrt(u+delta)) = trunc(sqrt(K^2*u+K^2*d))
                    nc.scalar.activation(o8[:, :w], u[:, :w], Act.Sqrt,
                                         bias=bias_t[:], scale=k2)
                else:
                    sf = gpool.tile([FP, TC], dt.float16, tag="g")
                    nc.scalar.activation(sf[:, :w], u[:, :w], Act.Ln,
                                         bias=bias_t[:], scale=k2)
                    nc.scalar.activation(o8[:, :w], sf[:, :w], Act.Exp,
                                         scale=float(r))
                nc.sync.dma_start(y[:, offs[k]:offs[k] + w], o8[:, :w])
                st[k].clear()

            # Software-pipelined emission: u runs one tile behind q/g and
            # sqrt+dma_out two tiles behind, so every op's inputs are ready
            # well before its engine dispatches it (no cross-engine
            # round-trip stalls in the in-order queues). DVE's share of the
            # steady scans is front-loaded into the ramp, where DVE would
            # otherwise idle; the tail then ends on Pool's last (small)
            # scan with a short drain chain.
            for j in DMA_ORDER:
                if j < N:
                    dma_in(j)
            scan(0)
            tile0_mid()  # q0, g0, u0 in one go
            for k in DVE_SCAN:
                if k < N:
                    scan(k)
            for k in range(1, N):
                if scan_on_pool(k):
                    scan(k)
                if k + 4 < N:
                    dma_in(k + 4)
                mid_qg(k)
                if k >= 2:
                    mid_u(k - 1)
                if k >= 2:
                    tail(k - 2)
            mid_u(N - 1)
            tail(N - 2)
            tail(N - 1)

    nc.compile()
    return nc


def _get_nc(alpha: float, r: float, delta: float):
    key = (round(alpha, 9), round(r, 9), round(delta, 9))
    if key not in _CACHE:
        _CACHE[key] = _build(alpha, r, delta)
    return _CACHE[key]


def _decode(o8: np.ndarray, r: float, delta: float) -> np.ndarray:
    return (o8.astype(np.float32) + np.float32(0.5)) / np.float32(K) \
        - np.float32(float(delta) ** float(r))


def _make_runner(nc):
    """Cached variant of bass2jax.run_bass_via_pjrt's multi-core branch.

    run_bass_kernel_spmd builds a fresh jax.jit closure per call (full
    retrace) and round-trips the full array through per-core split +
    concat. Since the 8 shards concatenated on axis 0 ARE the full
    [1024, 50000] array, we jit once and feed/return the full array
    directly.
    """
    import jax
    from jax.experimental.shard_map import shard_map
    from jax.sharding import Mesh, PartitionSpec
    from concourse import bass2jax

    bass2jax.install_neuronx_cc_hook()
    if nc.dbg_callbacks:
        raise RuntimeError("dbg callbacks unsupported in cached runner")
    partition_name = (nc.partition_id_tensor.name
                      if nc.partition_id_tensor else None)
    in_names, out_names, out_avals = [], [], []
    for alloc in nc.m.functions[0].allocations:
        if not isinstance(alloc, mybir.MemoryLocationSet):
            continue
        name = alloc.memorylocations[0].name
        if alloc.kind == "ExternalInput":
            if name != partition_name:
                in_names.append(name)
        elif alloc.kind == "ExternalOutput":
            out_names.append(name)
            out_avals.append(jax.core.ShapedArray(
                tuple(alloc.tensor_shape), mybir.dt.np(alloc.dtype)))
    extra_ins = {}
    if nc.dbg_addr is not None:
        extra_ins[nc.dbg_addr.name] = np.zeros((1, 2), np.uint32)
        if nc.dbg_addr.name not in in_names:
            in_names.append(nc.dbg_addr.name)
    assert in_names[0] == "x" and out_names == ["y"], (in_names, out_names)
    n_params = len(in_names)
    all_names = list(in_names) + list(out_names)
    if partition_name is not None:
        all_names.append(partition_name)
    donate = tuple(range(n_params, n_params + len(out_names)))

    def _body(*args):
        operands = list(args)
        if partition_name is not None:
            operands.append(bass2jax.partition_id_tensor())
        outs = bass2jax._bass_exec_p.bind(
            *operands,
            out_avals=tuple(out_avals),
            in_names=tuple(all_names),
            out_names=tuple(out_names),
            lowering_input_output_aliases=(),
            sim_require_finite=True,
            sim_require_nnan=True,
            nc=nc,
        )
        return tuple(outs)

    devices = jax.devices()[:NCORES]
    assert len(devices) == NCORES, devices
    mesh = Mesh(np.asarray(devices), ("core",))
    nio = n_params + len(out_names)
    sharded = jax.jit(
        shard_map(_body, mesh=mesh,
                  in_specs=(PartitionSpec("core"),) * nio,
                  out_specs=(PartitionSpec("core"),) * len(out_names),
                  check_rep=False),
        donate_argnums=donate, keep_unused=True)

    def run(data: np.ndarray) -> np.ndarray:
        extras = [np.concatenate([v] * NCORES, axis=0)
                  for v in extra_ins.values()]
        zeros = [np.zeros((NCORES * a.shape[0], *a.shape[1:]), a.dtype)
                 for a in out_avals]
        outs = sharded(data, *extras, *zeros)
        return np.asarray(outs[0])

    return run


def kernel(data, alpha=None, r=None, delta=None) -> np.ndarray:
    data = np.asarray(data)
    assert data.shape == (F, T), data.shape
    dh = np.ascontiguousarray(data.astype(np.float16))
    a = float(np.asarray(alpha).reshape(-1)[0]) if alpha is not None else 0.98
    rr = float(np.asarray(r).reshape(-1)[0]) if r is not None else 0.5
    d = float(np.asarray(delta).reshape(-1)[0]) if delta is not None else 2.0

    nc = _get_nc(a, rr, d)
    rkey = ("runner", round(a, 9), round(rr, 9), round(d, 9))
    try:
        if rkey not in _CACHE:
            _CACHE[rkey] = _make_runner(nc)
        o8 = _CACHE[rkey](dh)
    except Exception:  # fall back to the stock SPMD path
        _CACHE[rkey] = None
        in_maps = [{"x": dh[i * FP:(i + 1) * FP]} for i in range(NCORES)]
        res = run_bass_kernel_spmd(nc, in_maps, core_ids=list(range(NCORES)))
        o8 = np.concatenate([res.results[i]["y"] for i in range(NCORES)],
                            axis=0)
    return _decode(o8, rr, d)


# revision 24
# speedup vs baseline: 1.0651x; 1.0156x over previous
"""PCEN (per-channel energy normalization) Trainium2 Bass kernel, v2.

Problem: data [1024, 50000] f32, EMA along time (s=0.5) then
    out = (x / (EPS + M)**alpha + delta)**r - delta**r

Sharding: freq axis (dim 0) split across 8 NeuronCores, 128 rows/core.

v2 design (vs the v1 recip+table-switch pipeline):
  - With v2 = 2M from the native scan, q = x/v2 is computed by an exact
    DVE tensor_tensor divide (q <= ~1 always since v2 >= x), and
    u = x/(eps+M)^alpha = q * w(v2) where w(v2) = v2*(eps+v2/2)^-alpha
    spans only [1.66, 2.0]: affine-in-int16-bits fit, 0.3% max rel err
    over the steady-state v2 range [2e-3, 2.2].
  - No ACT Reciprocal at all -> zero activation-table switches; ACT runs
    Sqrt (+ Copy for part of the w fits) from one table set.
  - Output is written as uint8: o8 = trunc(K*sqrt(u+delta)) via a single
    pre-scaled ACT Sqrt (scale=K^2, bias=delta*K^2), K=127. The host
    decodes out = (o8+0.5)/K - delta^r. Quant err 0.5/127 = 3.9e-3 abs,
    well inside the 2e-2 budget, and it halves output DMA traffic.
  - Scans are made independent per tile with a 16-col zero-seeded halo
    (EMA forgets at 2^-t; the halo error is ~v2*2^-17), so the scan can
    be split across Pool (18 tiles) and DVE (3 tiles) with no serial
    carry chain.
  - Tile 0 (512 cols, where v2 can legitimately be tiny) runs an exact
    eps path: v = 0.5*v2 + eps in fp32, q0 = x/v (DVE divide), and
    g0 = v^(1-alpha) fitted affinely in the int32 bits of fp32 v over
    [5e-7, 1.2] (the 0.02 exponent makes this ~1% accurate, and u<=2
    there, so the output error stays ~2e-3).

Engine balance (per-core, TimelineSim rates): Pool 66us scan; DVE 67us
(scan 4 + div 27 + mult 27 + ~26k cols of w-fit at 4x); ACT 67us (sqrt
45 + ~23k cols of w-fit via Copy); DMA 58us (fp16 in + uint8 out).
"""

import numpy as np

import concourse.bass as bass
import concourse.bacc as bacc
import concourse.mybir as mybir
from concourse import tile
from concourse.bass_utils import run_bass_kernel_spmd

F, T = 1024, 50000
NCORES = 8
FP = F // NCORES  # 128 partitions per core
EPS = 1e-6
K = 127.0  # uint8 output scale

T0 = 512           # tile 0: exact-eps path
TC = 2560          # max steady tile width (buffer size)
HALO = 16          # zero-seeded scan warm-up cols per steady tile

# Schedule knobs (tuned against TimelineSim):
HEAD = (512, 1024)        # tile widths at the start (incl tile 0)
TAIL = (1536, 1024, 512)  # tile widths at the end
DVE_SCAN = (2, 3)         # steady tiles scanned on DVE (rest Pool)
ACT_G = (1, 3, 5, 7, 9, 11, 13, 15)  # tiles whose w-fit runs on ACT Copy
DMA_ORDER = (1, 0, 4, 2, 3)   # upfront in-DMA issue order
FRONTLOAD_DVE_SCAN = True  # emit DVE scans right after tile0 (else in-slot)
QSKEW = 0                  # 1 = q/g lag the scan front by one extra tile
DVE_MID_W = 2100           # width of the DVE-scanned mid tiles

_CACHE: dict = {}


def _tiles():
    mid = T - sum(HEAD) - sum(TAIL)
    # DVE-scanned mid tiles get width DVE_MID_W (DVE scans cost 1.066/col
    # vs Pool 1.427: narrower DVE tiles shift cols to Pool to equalize);
    # the remaining mids split the rest evenly.
    n_dve = len([k for k in DVE_SCAN if k >= len(HEAD)])
    rest = mid - n_dve * DVE_MID_W
    n_rest = max(1, -(-rest // TC))
    n_mid = n_rest + n_dve
    dve_mids = [k - len(HEAD) for k in DVE_SCAN
                if len(HEAD) <= k < len(HEAD) + n_mid]
    n_dve = len(dve_mids)
    base = rest // n_rest
    rem = rest - base * n_rest
    widths = [base + (1 if i < rem else 0) for i in range(n_rest)]
    mids = []
    for i in range(n_mid):
        mids.append(DVE_MID_W if i in dve_mids else widths.pop())
    assert all(0 < m + HALO <= TC + HALO for m in mids)
    tiles = list(HEAD) + mids + list(TAIL)
    assert sum(tiles) == T
    return tiles


def _irls_fit(codes, target):
    """Minimax-relative affine fit target ~ c1*codes + c0 via IRLS."""
    w = np.ones_like(target)
    co = np.polyfit(codes, target, 1, w=w / target)
    for _ in range(80):
        co = np.polyfit(codes, target, 1, w=w / target)
        rel = (np.polyval(co, codes) - target) / target
        w = (np.abs(rel) + 1e-7) * w
        w /= w.max()
    return float(co[0]), float(co[1])


def _fit_w_steady(alpha: float):
    """w(v2) = v2*(eps+v2/2)^-alpha, affine in int16 bits of fp16 v2,
    over the steady-state range [2e-3, 2.2]."""
    lo = np.float16(2e-3).view(np.int16)
    hi = np.float16(2.2).view(np.int16)
    codes = np.arange(int(lo), int(hi) + 1, dtype=np.int16)
    vals = codes.view(np.float16).astype(np.float64)
    keep = (vals > 0) & np.isfinite(vals)
    bc = codes[keep].astype(np.float64)
    vals = vals[keep]
    wi = vals * (EPS + 0.5 * vals) ** (-alpha)
    return _irls_fit(bc, wi)


def _fit_g0_tile0(alpha: float):
    """g0(v) = v^(1-alpha), affine in int32 bits of fp32 v over
    [5e-7, 1.2] (tile-0 exact-eps path; v = 0.5*v2 + eps)."""
    v = np.geomspace(5e-7, 1.2, 20000).astype(np.float32)
    bc = v.view(np.int32).astype(np.float64)
    gi = v.astype(np.float64) ** (1.0 - alpha)
    return _irls_fit(bc, gi)


def _build(alpha: float, r: float, delta: float):
    dt = mybir.dt
    Act = mybir.ActivationFunctionType
    Alu = mybir.AluOpType
    use_sqrt = abs(r - 0.5) < 1e-12
    c1, c0 = _fit_w_steady(alpha)
    d1, d0 = _fit_g0_tile0(alpha)
    k2 = float(K * K)

    nc = bacc.Bacc("TRN2", debug=False, enable_asserts=False,
                   target_bir_lowering=False)
    x = nc.dram_tensor("x", [FP, T], dt.float16, kind="ExternalInput").ap()
    y = nc.dram_tensor("y", [FP, T], dt.uint8, kind="ExternalOutput").ap()

    tiles = _tiles()
    N = len(tiles)
    offs = [0]
    for w in tiles:
        offs.append(offs[-1] + w)

    def scan_on_pool(k):
        return k >= 1 and k not in DVE_SCAN

    def g_on_act(k):
        return k in ACT_G

    with tile.TileContext(nc) as tc:
        with (
            tc.tile_pool(name="const", bufs=1) as cpool,
            tc.tile_pool(name="x", bufs=8) as xpool,
            tc.tile_pool(name="m", bufs=6) as mpool,
            tc.tile_pool(name="g", bufs=5) as gpool,
            tc.tile_pool(name="o", bufs=5) as opool,
            tc.tile_pool(name="t0", bufs=1) as t0pool,
        ):
            halfful = cpool.tile([FP, TC + HALO], dt.float16, tag="half")
            nc.vector.memset(halfful[:], 0.5)
            bias_t = cpool.tile([FP, 1], dt.float32, tag="bias")
            nc.gpsimd.memset(bias_t[:], float(delta) * k2)
            # Warm-up Sqrt with no data deps: pulls the one ACT table load
            # into the ramp.
            warm = cpool.tile([FP, 1], dt.float32, tag="warm")
            nc.scalar.activation(warm[:], bias_t[:],
                                 Act.Sqrt if use_sqrt else Act.Ln,
                                 bias=bias_t[:], scale=1.0)

            st: list[dict] = [dict() for _ in range(N)]

            def dma_in(k):
                w = tiles[k]
                if k == 0:
                    xt = xpool.tile([FP, TC + HALO], dt.float16, tag="x")
                    nc.sync.dma_start(xt[:, :w], x[:, :w])
                else:
                    xt = xpool.tile([FP, TC + HALO], dt.float16, tag="x")
                    nc.sync.dma_start(xt[:, :w + HALO],
                                      x[:, offs[k] - HALO:offs[k] + w])
                st[k]["x"] = xt

            def scan(k):
                w = tiles[k]
                wh = w if k == 0 else w + HALO
                m2 = mpool.tile([FP, TC + HALO], dt.float16, tag="m")
                eng = nc.gpsimd if scan_on_pool(k) else nc.vector
                eng.tensor_tensor_scan(
                    m2[:, :wh], halfful[:, :wh], st[k]["x"][:, :wh],
                    0.0, Alu.mult, Alu.add)
                st[k]["m"] = m2

            def tile0_mid():
                w = tiles[0]
                xt, m2 = st[0]["x"], st[0]["m"]
                vf = t0pool.tile([FP, T0], dt.float32, tag="v0")
                nc.vector.tensor_scalar(vf[:, :w], m2[:, :w], 0.5, EPS,
                                        op0=Alu.mult, op1=Alu.add)
                q = xt  # reuse x tile for q
                nc.vector.tensor_tensor(q[:, :w], xt[:, :w], vf[:, :w],
                                        Alu.divide)
                g = gpool.tile([FP, TC], dt.float16, tag="g")
                b32 = vf[:, :w].bitcast(dt.int32)
                nc.vector.tensor_scalar(g[:, :w], b32, d1, d0,
                                        op0=Alu.mult, op1=Alu.add)
                u = m2  # reuse m2 tile for u
                nc.vector.tensor_tensor(u[:, :w], q[:, :w], g[:, :w],
                                        Alu.mult)
                st[0]["u"] = u

            def mid_qg(k):
                w = tiles[k]
                xt, m2 = st[k]["x"], st[k]["m"]
                xs = xt[:, HALO:HALO + w]
                v2 = m2[:, HALO:HALO + w]
                q = xs  # reuse x tile for q (in place)
                nc.vector.tensor_tensor(q, xs, v2, Alu.divide)
                g = gpool.tile([FP, TC], dt.float16, tag="g")
                b16 = v2.bitcast(dt.int16)
                if g_on_act(k):
                    nc.scalar.activation(g[:, :w], b16, Act.Copy,
                                         bias=c0, scale=c1)
                else:
                    nc.vector.tensor_scalar(g[:, :w], b16, c1, c0,
                                            op0=Alu.mult, op1=Alu.add)
                st[k]["q"] = q
                st[k]["g"] = g

            def mid_u(k):
                w = tiles[k]
                g = st[k]["g"]
                u = g  # reuse g tile for u
                nc.vector.tensor_tensor(u[:, :w], st[k]["q"], g[:, :w],
                                        Alu.mult)
                st[k]["u"] = u

            def tail(k):
                w = tiles[k]
                u = st[k]["u"]
                o8 = opool.tile([FP, TC], dt.uint8, tag="o")
                if use_sqrt:
                    # o8 = trunc(K*sqrt(u+delta)) = trunc(sqrt(K^2*u+K^2*d))
                    nc.scalar.activation(o8[:, :w], u[:, :w], Act.Sqrt,
                                         bias=bias_t[:], scale=k2)
                else:
                    sf = gpool.tile([FP, TC], dt.float16, tag="g")
                    nc.scalar.activation(sf[:, :w], u[:, :w], Act.Ln,
                                         bias=bias_t[:], scale=k2)
                    nc.scalar.activation(o8[:, :w], sf[:, :w], Act.Exp,
                                         scale=float(r))
                nc.sync.dma_start(y[:, offs[k]:offs[k] + w], o8[:, :w])
                st[k].clear()

            # Software-pipelined emission: u runs one tile behind q/g and
            # sqrt+dma_out two tiles behind, so every op's inputs are ready
            # well before its engine dispatches it (no cross-engine
            # round-trip stalls in the in-order queues). DVE's share of the
            # steady scans is front-loaded into the ramp, where DVE would
            # otherwise idle; the tail then ends on Pool's last (small)
            # scan with a short drain chain.
            for j in DMA_ORDER:
                if j < N:
                    dma_in(j)
            scan(0)
            tile0_mid()  # q0, g0, u0 in one go
            if FRONTLOAD_DVE_SCAN:
                for k in DVE_SCAN:
                    if k < N:
                        scan(k)
            for k in range(1, N):
                if scan_on_pool(k) or not FRONTLOAD_DVE_SCAN:
                    scan(k)
                if k + 4 < N:
                    dma_in(k + 4)
                if QSKEW == 0:
                    if k >= 2:
                        mid_u(k - 1)  # ready work first: DVE never head-blocks
                    mid_qg(k)
                    if k >= 2:
                        tail(k - 2)
                else:
                    if k >= 2:
                        mid_qg(k - 1)
                    if k >= 3:
                        mid_u(k - 2)
                    if k >= 4:
                        tail(k - 3)
            if QSKEW == 0:
                mid_u(N - 1)
                tail(N - 2)
                tail(N - 1)
            else:
                mid_qg(N - 1)
                mid_u(N - 2)
                tail(N - 3)
                mid_u(N - 1)
                tail(N - 2)
                tail(N - 1)

    nc.compile()
    return nc


def _get_nc(alpha: float, r: float, delta: float):
    key = (round(alpha, 9), round(r, 9), round(delta, 9))
    if key not in _CACHE:
        _CACHE[key] = _build(alpha, r, delta)
    return _CACHE[key]


def _decode(o8: np.ndarray, r: float, delta: float) -> np.ndarray:
    return (o8.astype(np.float32) + np.float32(0.5)) / np.float32(K) \
        - np.float32(float(delta) ** float(r))


def _make_runner(nc):
    """Cached variant of bass2jax.run_bass_via_pjrt's multi-core branch.

    run_bass_kernel_spmd builds a fresh jax.jit closure per call (full
    retrace) and round-trips the full array through per-core split +
    concat. Since the 8 shards concatenated on axis 0 ARE the full
    [1024, 50000] array, we jit once and feed/return the full array
    directly.
    """
    import jax
    from jax.experimental.shard_map import shard_map
    from jax.sharding import Mesh, PartitionSpec
    from concourse import bass2jax

    bass2jax.install_neuronx_cc_hook()
    if nc.dbg_callbacks:
        raise RuntimeError("dbg callbacks unsupported in cached runner")
    partition_name = (nc.partition_id_tensor.name
                      if nc.partition_id_tensor else None)
    in_names, out_names, out_avals = [], [], []
    for alloc in nc.m.functions[0].allocations:
        if not isinstance(alloc, mybir.MemoryLocationSet):
            continue
        name = alloc.memorylocations[0].name
        if alloc.kind == "ExternalInput":
            if name != partition_name:
                in_names.append(name)
        elif alloc.kind == "ExternalOutput":
            out_names.append(name)
            out_avals.append(jax.core.ShapedArray(
                tuple(alloc.tensor_shape), mybir.dt.np(alloc.dtype)))
    extra_ins = {}
    if nc.dbg_addr is not None:
        extra_ins[nc.dbg_addr.name] = np.zeros((1, 2), np.uint32)
        if nc.dbg_addr.name not in in_names:
            in_names.append(nc.dbg_addr.name)
    assert in_names[0] == "x" and out_names == ["y"], (in_names, out_names)
    n_params = len(in_names)
    all_names = list(in_names) + list(out_names)
    if partition_name is not None:
        all_names.append(partition_name)
    donate = tuple(range(n_params, n_params + len(out_names)))

    def _body(*args):
        operands = list(args)
        if partition_name is not None:
            operands.append(bass2jax.partition_id_tensor())
        outs = bass2jax._bass_exec_p.bind(
            *operands,
            out_avals=tuple(out_avals),
            in_names=tuple(all_names),
            out_names=tuple(out_names),
            lowering_input_output_aliases=(),
            sim_require_finite=True,
            sim_require_nnan=True,
            nc=nc,
        )
        return tuple(outs)

    devices = jax.devices()[:NCORES]
    assert len(devices) == NCORES, devices
    mesh = Mesh(np.asarray(devices), ("core",))
    nio = n_params + len(out_names)
    sharded = jax.jit(
        shard_map(_body, mesh=mesh,
                  in_specs=(PartitionSpec("core"),) * nio,
                  out_specs=(PartitionSpec("core"),) * len(out_names),
                  check_rep=False),
        donate_argnums=donate, keep_unused=True)

    def run(data: np.ndarray) -> np.ndarray:
        extras = [np.concatenate([v] * NCORES, axis=0)
                  for v in extra_ins.values()]
        zeros = [np.zeros((NCORES * a.shape[0], *a.shape[1:]), a.dtype)
                 for a in out_avals]
        outs = sharded(data, *extras, *zeros)
        return np.asarray(outs[0])

    return run


def kernel(data, alpha=None, r=None, delta=None) -> np.ndarray:
    data = np.asarray(data)
    assert data.shape == (F, T), data.shape
    dh = np.ascontiguousarray(data.astype(np.float16))
    a = float(np.asarray(alpha).reshape(-1)[0]) if alpha is not None else 0.98
    rr = float(np.asarray(r).reshape(-1)[0]) if r is not None else 0.5
    d = float(np.asarray(delta).reshape(-1)[0]) if delta is not None else 2.0

    nc = _get_nc(a, rr, d)
    rkey = ("runner", round(a, 9), round(rr, 9), round(d, 9))
    try:
        if rkey not in _CACHE:
            _CACHE[rkey] = _make_runner(nc)
        o8 = _CACHE[rkey](dh)
    except Exception:  # fall back to the stock SPMD path
        _CACHE[rkey] = None
        in_maps = [{"x": dh[i * FP:(i + 1) * FP]} for i in range(NCORES)]
        res = run_bass_kernel_spmd(nc, in_maps, core_ids=list(range(NCORES)))
        o8 = np.concatenate([res.results[i]["y"] for i in range(NCORES)],
                            axis=0)
    return _decode(o8, rr, d)
